# revision 6
# baseline (speedup 1.0000x reference)
"""Trainium2 Bass kernel for per-sample expert-routed 2-layer MLP (MoE routing).

Problem: logits[b] = relu(V[b] @ W1[id[b]] + b1[id[b]]) @ W2[id[b]] + b2[id[b]]
  V = concat(v_X, v_H): (256, 1536), 32 experts, W1 per expert (1536, 768).

Strategy (expert parallel over 8 NeuronCores):
  - Host routes samples to experts (cheap; 256 int compares) and assigns
    4 experts to each core. Each expert's W1 (4.7 MB) is streamed from HBM
    exactly once chip-wide: 18.9 MB per core, the memory-roofline minimum.
  - Per expert: out1 = relu(V_e @ W1_e + b1_e) computed with V_e^T as the
    PE-stationary operand and W1_e as the moving operand (float32r: full
    1 col/cycle rate at N>=256), bias folded in as an extra K=1 matmul
    against an all-ones lhsT row. Layer 2 (768 -> 2) runs on the vector
    engine as two dot-product reductions against host-broadcast W2 columns
    (with b2 folded in via an appended ones column).
  - Outputs (capacity-padded per-expert logits) are scattered back on host.
"""

import numpy as np

import concourse.bacc as bacc
import concourse.mybir as mybir
import concourse.tile as tile
from concourse.bass_utils import run_bass_kernel_spmd

N_CORES = 8
KT = 12          # K tiles of 128 over D=1536
D = 1536
H = 768

_graph_cache = {}


def _build(G: int, C: int):
    """Build the SPMD graph: G expert-groups per core, capacity C samples."""
    dt = mybir.dt
    Act = mybir.ActivationFunctionType
    Alu = mybir.AluOpType

    nc = bacc.Bacc("TRN2", target_bir_lowering=False, debug=False,
                   enable_asserts=False)

    w1d = nc.dram_tensor("w1", [G, KT, 128, H], dt.float32r, kind="ExternalInput")
    vtd = nc.dram_tensor("vt", [128, G, KT + 1, C], dt.float32r, kind="ExternalInput")
    b1d = nc.dram_tensor("b1r", [1, G, H], dt.float32r, kind="ExternalInput")
    w2d = nc.dram_tensor("w2e", [C, G, 2, H + 1], dt.float32, kind="ExternalInput")
    outd = nc.dram_tensor("out", [C, G * 2], dt.float32, kind="ExternalOutput")

    with tile.TileContext(nc) as tc:
        with (
            tc.tile_pool(name="const", bufs=1) as constp,
            tc.tile_pool(name="w1p", bufs=2) as w1p,
            tc.tile_pool(name="work", bufs=2) as workp,
            tc.tile_pool(name="psum", bufs=2, space="PSUM") as psump,
        ):
            vt = constp.tile([128, G, KT + 1, C], dt.float32r)
            b1 = constp.tile([1, G, H], dt.float32r)
            w2 = constp.tile([C, G, 2, H + 1], dt.float32)
            logits = constp.tile([C, G * 2], dt.float32)
            nc.sync.dma_start(vt[:], vtd[:])
            nc.sync.dma_start(b1[:], b1d[:])
            nc.sync.dma_start(w2[:], w2d[:])

            for g in range(G):
                w1t = w1p.tile([128, KT, H], dt.float32r, tag="w1")
                nc.sync.dma_start(w1t[:], w1d[g].rearrange("k p h -> p k h"))

                ps = psump.tile([C, H], dt.float32, tag="ps")
                for lo, hi in ((0, 512), (512, H)):
                    for k in range(KT):
                        nc.tensor.matmul(
                            ps[:, lo:hi], vt[:, g, k, :], w1t[:, k, lo:hi],
                            start=(k == 0), stop=False,
                        )
                    # bias: K=1 matmul, all-ones lhsT row times b1 row
                    nc.tensor.matmul(
                        ps[:, lo:hi], vt[0:1, g, KT, :], b1[0:1, g, lo:hi],
                        start=False, stop=True,
                    )

                o1 = workp.tile([C, H + 1], dt.float32, tag="o1")
                nc.scalar.activation(o1[:, 0:512], ps[:, 0:512], Act.Relu)
                nc.scalar.activation(o1[:, 512:H], ps[:, 512:H], Act.Relu)
                nc.vector.memset(o1[:, H:H + 1], 1.0)

                for t in range(2):
                    scr = workp.tile([C, H + 1], dt.float32, tag="scr")
                    nc.vector.tensor_mul(scr[:], o1[:], w2[:, g, t, :])
                    nc.vector.reduce_sum(
                        logits[:, g * 2 + t:g * 2 + t + 1], scr[:],
                        axis=mybir.AxisListType.X,
                    )

            nc.sync.dma_start(outd[:], logits[:])

    nc.compile()
    return nc


def _route(ids: np.ndarray, n_experts: int):
    """Group sample indices by expert; split groups >128; pad count to 8k."""
    groups = []
    for e in range(n_experts):
        idx = np.nonzero(ids == e)[0]
        if len(idx) <= 128:
            groups.append((e, idx))
        else:
            for j in range(0, len(idx), 128):
                groups.append((e, idx[j:j + 128]))
    while len(groups) % N_CORES:
        groups.append((0, np.empty(0, np.int64)))
    G = len(groups) // N_CORES
    maxn = max(max((len(i) for _, i in groups)), 1)
    C = ((maxn + 31) // 32) * 32
    return groups, G, C


def _run(inputs: dict, trace: bool = False, **run_kwargs):
    v_X = np.asarray(inputs["v_X"], dtype=np.float32)
    v_H = np.asarray(inputs["v_H"], dtype=np.float32)
    ids = np.asarray(inputs["aspect_ids"]).astype(np.int64)
    W1 = np.asarray(inputs["W1_embs"], dtype=np.float32)
    b1 = np.asarray(inputs["b1_embs"], dtype=np.float32)
    W2 = np.asarray(inputs["W2_embs"], dtype=np.float32)
    b2 = np.asarray(inputs["b2_embs"], dtype=np.float32)

    B = v_X.shape[0]
    A = W1.shape[0]
    V = np.concatenate([v_X, v_H], axis=1)  # (B, D)
    assert V.shape[1] == D and b1.shape[1] == H

    groups, G, C = _route(ids, A)

    key = (G, C)
    if key not in _graph_cache:
        _graph_cache[key] = _build(G, C)
    nc = _graph_cache[key]

    in_maps = []
    for c in range(N_CORES):
        cg = groups[c * G:(c + 1) * G]
        w1c = np.stack([W1[e].reshape(KT, 128, H) for e, _ in cg])
        vtc = np.zeros((128, G, KT + 1, C), dtype=np.float32)
        w2c = np.zeros((C, G, 2, H + 1), dtype=np.float32)
        b1c = np.stack([b1[e] for e, _ in cg])[None]  # (1, G, H)
        for g, (e, idx) in enumerate(cg):
            n = len(idx)
            if n:
                # V[idx].T: (D, n) -> (KT, 128, n) -> [p, k, c]
                vtc[:, g, :KT, :n] = V[idx].T.reshape(KT, 128, n).transpose(1, 0, 2)
            vtc[0, g, KT, :] = 1.0
            w2r = W2[e].reshape(H, 2)
            w2c[:, g, 0, :H] = w2r[:, 0]
            w2c[:, g, 1, :H] = w2r[:, 1]
            w2c[:, g, 0, H] = b2[e, 0]
            w2c[:, g, 1, H] = b2[e, 1]
        in_maps.append({
            "w1": np.ascontiguousarray(w1c),
            "vt": np.ascontiguousarray(vtc),
            "b1r": np.ascontiguousarray(b1c),
            "w2e": np.ascontiguousarray(w2c),
        })

    res = run_bass_kernel_spmd(nc, in_maps, core_ids=list(range(N_CORES)),
                               trace=trace, **run_kwargs)

    logits = np.zeros((B, 2), dtype=np.float32)
    for c in range(N_CORES):
        out_c = res.results[c]["out"]  # (C, G*2)
        for g, (e, idx) in enumerate(groups[c * G:(c + 1) * G]):
            n = len(idx)
            if n:
                logits[idx, 0] = out_c[:n, 2 * g]
                logits[idx, 1] = out_c[:n, 2 * g + 1]
    return logits, res


def kernel(**inputs) -> np.ndarray:
    logits, _ = _run(inputs, trace=False)
    return logits
